# revision 45
# baseline (speedup 1.0000x reference)
"""Trainium2 Bass kernel for 16-head self-attention (N=4, S=2048, E=1024).

Sharding: 8 cores = 4 batches x 2 head-groups (8 heads each).  Each core
computes its head-group's attention and a partial fc_out product
(S x 1024, bf16); the host sums the two partials per batch and adds the
bias in f32.

Design (per core) — the exp stream is the throughput floor, so the kernel
is a uniform software pipeline that splits it across two engines and keeps
both saturated:

  - 16 "steps", one per (query-chunk cc, head-pair pr).  Step s emits
    scores+exp for step s and the ctx matmuls for step s-1 (ctx trails
    exp by one step so the PE queue never blocks the exp producers).
  - scoresT = K.T-blocks (stationary) @ Q.T (moving); the two heads of a
    pair row-tile into disjoint 64-row PE groups and run concurrently.
  - exp: 11/16 key blocks on ScalarE (LUT exp, scale=1/sqrt(E) fused),
    5/16 on VectorE via the Schraudolph bit trick (one fused
    tensor_scalar: int16(x*128/ln2 + b) bitcast to bf16, ~+-4.5%/elem,
    zero-mean; end-to-end max rel err 0.0136 vs the 0.02 gate) — both
    read the PSUM scores tiles concurrently.
  - ctx matmul lhsT = [Vproj | ones] (M=65): row 64 accumulates sumexp.
  - normalization is split in two phases so nothing stalls: phase A (end
    of step): PSUM->SBUF copy, SBUF->SBUF DMA gather of the sum rows to
    partition 0, reciprocal_approx_fast, stage to DRAM; phase B (mid next
    step): partition-broadcast DMA + multiply into the bf16 ctxT tile.
  - fc_out: bf16 matmuls accumulating over 4 head-pair blocks, drip-fed
    into later steps' jb streams; output cast to bf16 (host sums the two
    partial products per batch in f32) and DMAed.
  - input DMAs are ordered by first use and spread over the sync/gpsimd
    HWDGE queues (first-needed chunks first; one combined weights DMA);
    the scalar queue carries no DMA until the exp stream ends.
"""

import numpy as np
import ml_dtypes

EMBED = 1024
HEADS = 16
HD = 64  # head dim
N_CORES = 8
HPC = 8  # heads per core
GCOLS = HPC * HD  # 512 embed columns per core


def ts(i, n):
    return slice(i * n, (i + 1) * n)


def build_program(S):
    import concourse.bass as bass
    import concourse.tile as tile
    import concourse.mybir as mybir
    from concourse import bacc

    from concourse.alu_op_type import AluOpType

    f32 = mybir.dt.float32
    bf16 = mybir.dt.bfloat16
    i16 = mybir.dt.int16
    EXP = mybir.ActivationFunctionType.Exp

    NPAIR = 4  # head-pair blocks (2 heads each)
    NJB = S // 128  # key blocks
    NCCS = S // 512  # 512-wide query chunks
    NSTEP = NCCS * NPAIR  # (cc, pr) steps

    # Every scores tile is split between both exp engines: ScalarE runs the
    # LUT exp on one head's 512 columns while VectorE runs the Schraudolph
    # bit trick (bf16(exp(x)) ~= bitcast_bf16(int16(x*128/ln2 + 127*128-c)),
    # ~+-4.5%/elem, zero-mean) on the other head's.  The halves alternate
    # per key block, so each (head, query) sees the approximation on only
    # 8/16 key blocks: numpy sim on the real inputs gives max rel err 0.0151
    # vs the 0.02 gate.  Both engines consuming each PSUM scores tile
    # concurrently nearly halves the slot-release latency that was embedding
    # waits into the PE's matmul stream.
    SCH_A = float(128.0 / np.log(2.0))
    SCH_B = float(127.0 * 128.0 - 7.4)

    nc = bacc.Bacc("TRN2", target_bir_lowering=False, debug=False)

    xqT_d = nc.dram_tensor("xqT", [NPAIR, 128, S], bf16, kind="ExternalInput").ap()
    xkT_d = nc.dram_tensor("xkT", [NPAIR, 128, S], bf16, kind="ExternalInput").ap()
    xvT_d = nc.dram_tensor("xvT", [NPAIR, 128, S], bf16, kind="ExternalInput").ap()
    wqkv_d = nc.dram_tensor("wqkv", [128, 3 * HD], bf16, kind="ExternalInput").ap()
    wo_d = nc.dram_tensor("woT", [NPAIR, 128, EMBED], bf16, kind="ExternalInput").ap()
    out_d = nc.dram_tensor("out", [S, EMBED], bf16, kind="ExternalOutput").ap()
    rec_dram = nc.dram_tensor("rec_scratch", [NPAIR, 2, S], f32)

    with tile.TileContext(nc) as tc:
        import contextlib

        with contextlib.ExitStack() as ctx:
            # ---- persistent pools ----
            const_p = ctx.enter_context(tc.tile_pool(name="const", bufs=1))
            qkv_p = ctx.enter_context(tc.tile_pool(name="qkv", bufs=1))
            vp_p = ctx.enter_context(tc.tile_pool(name="vp", bufs=1))
            ctxT_p = ctx.enter_context(tc.tile_pool(name="ctxT", bufs=1))
            wo_p = ctx.enter_context(tc.tile_pool(name="wo", bufs=1))
            # PSUM: "sc" slots 2 banks each (x2 = 4 banks), "cps" ctx
            # accumulators 1 bank (x2), "tr" transients 1 bank (x2) -> 8.
            sc_ps = ctx.enter_context(tc.tile_pool(name="sc", bufs=2, space="PSUM"))
            cps_ps = ctx.enter_context(tc.tile_pool(name="cps", bufs=2, space="PSUM"))
            tr_ps = ctx.enter_context(tc.tile_pool(name="tr", bufs=2, space="PSUM"))

            exp_p = ctx.enter_context(tc.tile_pool(name="exp", bufs=24))
            rec_p = ctx.enter_context(tc.tile_pool(name="rec", bufs=1))
            cu_p = ctx.enter_context(tc.tile_pool(name="cu", bufs=4))
            rrs_p = ctx.enter_context(tc.tile_pool(name="rrs", bufs=2))
            fco_p = ctx.enter_context(tc.tile_pool(name="fco", bufs=2))

            inv_sqrt_e = 1.0 / float(np.sqrt(EMBED))

            # ---- weights: one combined DMA (each trigger costs ~600ns of
            # queue time, so fewer, earlier triggers -> earlier first exp) --
            wqkv_s = const_p.tile([128, 3 * HD], bf16, tag="wqkv")
            nc.sync.dma_start(wqkv_s[:], wqkv_d[:])
            wq_s = wqkv_s[:, 0 * HD : 1 * HD]
            wk_s = wqkv_s[:, 1 * HD : 2 * HD]
            wv_s = wqkv_s[:, 2 * HD : 3 * HD]
            wo_t = [wo_p.tile([128, EMBED], bf16, tag=f"wo{p}", name=f"wo{p}") for p in range(NPAIR)]
            qT = [qkv_p.tile([128, S], bf16, tag=f"qT{p}", name=f"qT{p}") for p in range(NPAIR)]
            kT = [qkv_p.tile([128, S], bf16, tag=f"kT{p}", name=f"kT{p}") for p in range(NPAIR)]
            vp_t = [vp_p.tile([128, HPC * 65], bf16, tag=f"vp{jb}", name=f"vp{jb}") for jb in range(NJB)]
            ctxT = [ctxT_p.tile([128, S], bf16, tag=f"cx{p}", name=f"cx{p}") for p in range(NPAIR)]

            with tc.tile_pool(name="xin", bufs=1) as xin_p:
                xq = [xin_p.tile([128, S], bf16, tag=f"xq{p}", name=f"xq{p}") for p in range(NPAIR)]
                xk = [xin_p.tile([128, S], bf16, tag=f"xk{p}", name=f"xk{p}") for p in range(NPAIR)]
                xv = [xin_p.tile([128, S], bf16, tag=f"xv{p}", name=f"xv{p}") for p in range(NPAIR)]
                # DMA order = first use, packed so every piece lands before
                # the in-order engine queues reach its consumer.  Only the
                # sync and gpsimd queues carry input DMAs; the scalar queue
                # stays free for the exp stream.
                C = 512
                nc.sync.dma_start(xq[0][:, 0:C], xqT_d[0][:, 0:C])
                nc.gpsimd.dma_start(xk[0][:, 0:C], xkT_d[0][:, 0:C])
                nc.gpsimd.dma_start(xv[0][:], xvT_d[0])
                nc.sync.dma_start(xk[0][:, C:S], xkT_d[0][:, C:S])
                nc.gpsimd.dma_start(xv[1][:], xvT_d[1])
                nc.sync.dma_start(xq[1][:, 0:C], xqT_d[1][:, 0:C])
                nc.gpsimd.dma_start(xk[1][:], xkT_d[1])
                nc.sync.dma_start(xv[2][:], xvT_d[2])
                nc.gpsimd.dma_start(xv[3][:], xvT_d[3])
                nc.sync.dma_start(xq[2][:, 0:C], xqT_d[2][:, 0:C])
                nc.gpsimd.dma_start(xk[2][:], xkT_d[2])
                nc.sync.dma_start(xq[3][:, 0:C], xqT_d[3][:, 0:C])
                nc.gpsimd.dma_start(xk[3][:], xkT_d[3])
                for p in range(NPAIR):
                    nc.sync.dma_start(xq[p][:, C:S], xqT_d[p][:, C:S])
                for p in range(NPAIR):
                    (nc.sync if p % 2 else nc.gpsimd).dma_start(wo_t[p][:], wo_d[p])

                # ---- Q/K projection piece: qT[p][:, chunk] etc. ----
                def emit_qk(w_s, x_t, dst, p, ch, pool, tag):
                    ps = pool.tile([128, 512], f32, tag=tag)
                    for b in (0, 64):
                        nc.tensor.matmul(
                            ps[b : b + 64, 0:512],
                            lhsT=w_s[b : b + 64, :],
                            rhs=x_t[p][b : b + 64, ts(ch, 512)],
                            start=True,
                            stop=True,
                        )
                    nc.vector.tensor_copy(dst[p][:, ts(ch, 512)], ps[:, 0:512])

                # V-projection for one key block, one pair-half (pairs 2h..)
                # -> vp tile [V | ones] per head.  Split by half so the
                # early pieces only need xv0/xv1.
                def emit_vproj(jb, half):
                    pse = tr_ps.tile([128, 128], f32, tag="tr")
                    pso = tr_ps.tile([128, 128], f32, tag="tr")
                    for hh in range(4):
                        h = half * 4 + hh
                        p, b = h // 2, (h % 2) * 64
                        dst = pse if h % 2 == 0 else pso
                        nc.tensor.matmul(
                            dst[:, ts(hh // 2, 64)],
                            lhsT=xv[p][b : b + 64, ts(jb, 128)],
                            rhs=wv_s[b : b + 64, :],
                            start=True,
                            stop=True,
                        )
                    vpr = vp_t[jb].rearrange("p (h2 two c) -> p h2 two c", two=2, c=65)
                    h2s = slice(half * 2, half * 2 + 2)
                    pse_r = pse.rearrange("p (h c) -> p h c", c=64)
                    pso_r = pso.rearrange("p (h c) -> p h c", c=64)
                    nc.vector.tensor_copy(vpr[:, h2s, 0, 0:64], pse_r[:])
                    nc.vector.tensor_copy(vpr[:, h2s, 1, 0:64], pso_r[:])
                    nc.vector.memset(vpr[:, h2s, :, 64], 1.0)

                # drip list with deadlines (step whose scores/ctx need the
                # piece).  Ordered so each piece's input DMA has landed by
                # its emission slot; flushed before each step as a backstop.
                drip = []  # (deadline_step, emit_fn)
                # vproj half 0 (pairs 0/1, xv0/xv1): dripped in warm
                drip += [(1, lambda jb=jb: emit_vproj(jb, 0)) for jb in range(NJB)]
                for ch in range(4):
                    drip.append((1, lambda ch=ch: emit_qk(wk_s, xk, kT, 1, ch, tr_ps, "tr")))
                drip.append((1, lambda: emit_qk(wq_s, xq, qT, 1, 0, tr_ps, "tr")))
                # (vproj half 1 is emitted inline in the s==1 jb loop)
                for p in (2, 3):
                    for ch in range(4):
                        drip.append((p, lambda p=p, ch=ch: emit_qk(wk_s, xk, kT, p, ch, tr_ps, "tr")))
                    drip.append((p, lambda p=p: emit_qk(wq_s, xq, qT, p, 0, tr_ps, "tr")))
                for ch in (1, 2, 3):
                    for p in range(NPAIR):
                        drip.append((ch * 4 + p, lambda p=p, ch=ch: emit_qk(wq_s, xq, qT, p, ch, tr_ps, "tr")))

                # ---- scores+exp for one (step, jb) ----
                e_tiles = {}

                def emit_scores_exp(s, jb):
                    cc, pr = s // NPAIR, s % NPAIR
                    s_t = sc_ps.tile([128, 1024], f32, tag="sc")
                    for hl, b in ((0, 0), (1, 64)):
                        nc.tensor.matmul(
                            s_t[:, ts(hl, 512)],
                            lhsT=kT[pr][b : b + 64, ts(jb, 128)],
                            rhs=qT[pr][b : b + 64, ts(cc, 512)],
                            start=True,
                            stop=True,
                        )
                    e_t = exp_p.tile([128, 1024], bf16, tag="exp")
                    dve_hl = jb % 2
                    act_hl = 1 - dve_hl
                    nc.scalar.activation(
                        e_t[:, ts(act_hl, 512)],
                        s_t[:, ts(act_hl, 512)],
                        EXP,
                        scale=inv_sqrt_e,
                    )
                    nc.vector.tensor_scalar(
                        e_t[:, ts(dve_hl, 512)].bitcast(i16),
                        s_t[:, ts(dve_hl, 512)],
                        SCH_A * inv_sqrt_e,
                        SCH_B,
                        AluOpType.mult,
                        AluOpType.add,
                    )
                    e_tiles[(s, jb)] = e_t

                # ---- warm: scores+exp for step 0, k-projections emitted
                # per chunk-group so the first exp starts as soon as the
                # first k/q chunk DMAs land.  Drip from jb>=8 (~16us in) so
                # the dripped work's inputs (xv0/1, xk1) have landed by the
                # time the in-order engine queues reach it.
                di = 0
                emit_qk(wq_s, xq, qT, 0, 0, sc_ps, "sc")
                for cg in range(4):
                    emit_qk(wk_s, xk, kT, 0, cg, sc_ps, "sc")
                    for jb in range(cg * 4, cg * 4 + 4):
                        emit_scores_exp(0, jb)
                        if 8 <= jb < 15:
                            for _ in range(3):
                                if di < len(drip):
                                    drip[di][1]()
                                    di += 1

                # ---- fc_out emission (one 512-col output block) ----
                def emit_fco(sb, oc, eng=None):
                    ps = tr_ps.tile([128, 512], f32, tag="tr")
                    for p in range(NPAIR):
                        nc.tensor.matmul(
                            ps[:],
                            lhsT=ctxT[p][:, ts(sb, 128)],
                            rhs=wo_t[p][:, ts(oc, 512)],
                            start=(p == 0),
                            stop=(p == NPAIR - 1),
                        )
                    fo = fco_p.tile([128, 512], bf16, tag="fco")
                    nc.scalar.copy(fo[:], ps[:])
                    (eng or nc.sync).dma_start(out_d[ts(sb, 128), ts(oc, 512)], fo[:])

                # ---- normalization, split in two phases so neither the PE
                # nor the Act queue ever waits on the reciprocal chain.
                # Phase A (end of the step that finished the accumulators):
                # copy ctx+sum to SBUF, gather sum rows to partition 0,
                # reciprocal, stage to DRAM for the partition-broadcast.
                # Phase B (mid next step): broadcast-DMA + multiply.
                def emit_norm_a(s, cps_pair):
                    se_c = rec_p.tile([1, 1024], f32, tag="se")
                    rec_c = rec_p.tile([1, 1024], f32, tag="rec")
                    pr, cc = s % NPAIR, s // NPAIR
                    cus = []
                    for hl, cps in ((0, cps_pair[0]), (1, cps_pair[1])):
                        cu = cu_p.tile([65, 512], f32, tag="cu")
                        nc.scalar.copy(cu[:], cps[:])
                        nc.sync.dma_start(se_c[0:1, ts(hl, 512)], cu[64:65, :])
                        cus.append(cu)
                    nc.vector.reciprocal_approx_fast(rec_c[:], se_c[:])
                    nc.sync.dma_start(rec_dram[pr][:, ts(cc, 512)], rec_c[:])
                    return cus

                def emit_norm_b(s, cus):
                    cc, pr = s // NPAIR, s % NPAIR
                    for hl in range(2):
                        rrs_c = rrs_p.tile([64, 512], f32, tag="rrs")
                        nc.sync.dma_start(
                            rrs_c[:],
                            rec_dram[pr][hl : hl + 1, ts(cc, 512)].partition_broadcast(64),
                        )
                        nc.vector.tensor_mul(
                            ctxT[pr][hl * 64 : hl * 64 + 64, ts(cc, 512)],
                            cus[hl][0:64, :],
                            rrs_c[:],
                        )

                # ---- main pipeline: step s emits exp(s) + ctx(s-1),
                # norm phase B of step s-2, and dripped proj/fc_out work ----
                fco_q = []
                normb = None  # (step, cus) pending phase B
                for s in range(1, NSTEP + 1):
                    # backstop: flush drip pieces this step's scores/ctx need
                    while di < len(drip) and drip[di][0] <= s:
                        drip[di][1]()
                        di += 1
                    pcc, ppr = (s - 1) // NPAIR, (s - 1) % NPAIR
                    cpsA = cps_ps.tile([65, 512], f32, tag="cps", name=f"cpsA{s - 1}")
                    cpsB = cps_ps.tile([65, 512], f32, tag="cps", name=f"cpsB{s - 1}")
                    for jb in range(NJB):
                        if s < NSTEP:
                            emit_scores_exp(s, jb)
                        if s == 1:
                            emit_vproj(jb, 1)
                        e_t = e_tiles.pop((s - 1, jb))
                        for hl, cps in ((0, cpsA), (1, cpsB)):
                            hh = ppr * 2 + hl
                            nc.tensor.matmul(
                                cps[:],
                                lhsT=vp_t[jb][:, hh * 65 : hh * 65 + 65],
                                rhs=e_t[:, ts(hl, 512)],
                                start=(jb == 0),
                                stop=(jb == NJB - 1),
                            )
                        if jb == 2 and normb is not None:
                            emit_norm_b(*normb)
                            if normb[0] % NPAIR == NPAIR - 1:
                                fcc = normb[0] // NPAIR
                                fco_q += [(fcc * 4 + g // 2, g % 2) for g in range(8)]
                            normb = None
                        if jb % 3 == 2:
                            if di < len(drip):
                                drip[di][1]()
                                di += 1
                            elif fco_q:
                                emit_fco(*fco_q.pop(0))
                            if fco_q and jb % 6 == 5:
                                emit_fco(*fco_q.pop(0))
                    cus = emit_norm_a(s - 1, (cpsA, cpsB))
                    normb = (s - 1, cus)
                # drain: phase B of the last step, then its fc_out chunk with
                # output DMA spread across sync/scalar/gpsimd queues
                emit_norm_b(*normb)
                fco_q += [((NCCS - 1) * 4 + g // 2, g % 2) for g in range(8)]
                engs = [nc.sync, nc.scalar, nc.gpsimd]
                for i, (sb, oc) in enumerate(fco_q):
                    emit_fco(sb, oc, engs[i % 3])

    nc.compile()
    return nc


def make_core_inputs(values, keys, queries, Wv, Wk, Wq, Wo, n, g, S):
    """Host-side marshaling for core (n, g): transpose + cast input slices."""
    bf = ml_dtypes.bfloat16
    cols = slice(g * GCOLS, (g + 1) * GCOLS)
    NPAIR = 4

    def xt(x):
        t = np.ascontiguousarray(x[n][:, cols].T.astype(bf))  # (512, S)
        return t.reshape(NPAIR, 128, S)

    def wstack(w):
        wt = w.T.astype(bf)  # (64, 64)
        return np.ascontiguousarray(np.concatenate([wt, wt], axis=0))  # (128, 64)

    woT = np.ascontiguousarray(Wo[:, cols].T.astype(bf)).reshape(
        NPAIR, 128, EMBED
    )
    return {
        "xqT": xt(queries),
        "xkT": xt(keys),
        "xvT": xt(values),
        "wqkv": np.ascontiguousarray(
            np.concatenate([wstack(Wq), wstack(Wk), wstack(Wv)], axis=1)
        ),
        "woT": woT,
    }


_PROG_CACHE = {}
TRACE = False
LAST_RESULTS = None


def kernel(values, keys, queries, mask, Wv, Wk, Wq, Wo, bo):
    global LAST_RESULTS
    from concourse.bass_utils import run_bass_kernel_spmd

    values = np.asarray(values, np.float32)
    keys = np.asarray(keys, np.float32)
    queries = np.asarray(queries, np.float32)
    Wv = np.asarray(Wv, np.float32)
    Wk = np.asarray(Wk, np.float32)
    Wq = np.asarray(Wq, np.float32)
    Wo = np.asarray(Wo, np.float32)
    bo = np.asarray(bo, np.float32)

    N, S, _ = queries.shape
    if S not in _PROG_CACHE:
        _PROG_CACHE[S] = build_program(S)
    nc = _PROG_CACHE[S]

    in_maps = [
        make_core_inputs(values, keys, queries, Wv, Wk, Wq, Wo, c // 2, c % 2, S)
        for c in range(N_CORES)
    ]
    res = run_bass_kernel_spmd(
        nc, in_maps, core_ids=list(range(N_CORES)), trace=TRACE
    )
    LAST_RESULTS = res
    out = np.empty((N, S, EMBED), np.float32)
    for n in range(N):
        out[n] = (
            res.results[2 * n]["out"].astype(np.float32)
            + res.results[2 * n + 1]["out"].astype(np.float32)
            + bo
        )
    return out


# revision 46
# speedup vs baseline: 1.0904x; 1.0904x over previous
"""Trainium2 Bass kernel for 16-head self-attention (N=4, S=2048, E=1024).

Sharding: 8 cores = 4 batches x 2 head-groups (8 heads each).  Each core
computes its head-group's attention and a partial fc_out product
(S x 1024, bf16); the host sums the two partials per batch and adds the
bias in f32.

Design (per core) — the exp stream is the throughput floor, so the kernel
is a uniform software pipeline that splits it across two engines and keeps
both saturated:

  - 16 "steps", one per (query-chunk cc, head-pair pr).  Step s emits
    scores+exp for step s and the ctx matmuls for step s-1 (ctx trails
    exp by one step so the PE queue never blocks the exp producers).
  - scoresT = K.T-blocks (stationary) @ Q.T (moving); the two heads of a
    pair row-tile into disjoint 64-row PE groups and run concurrently.
  - exp: 11/16 key blocks on ScalarE (LUT exp, scale=1/sqrt(E) fused),
    5/16 on VectorE via the Schraudolph bit trick (one fused
    tensor_scalar: int16(x*128/ln2 + b) bitcast to bf16, ~+-4.5%/elem,
    zero-mean; end-to-end max rel err 0.0136 vs the 0.02 gate) — both
    read the PSUM scores tiles concurrently.
  - ctx matmul lhsT = [Vproj | ones] (M=65): row 64 accumulates sumexp.
  - normalization is split in two phases so nothing stalls: phase A (end
    of step): PSUM->SBUF copy, SBUF->SBUF DMA gather of the sum rows to
    partition 0, reciprocal_approx_fast, stage to DRAM; phase B (mid next
    step): partition-broadcast DMA + multiply into the bf16 ctxT tile.
  - fc_out: bf16 matmuls accumulating over 4 head-pair blocks, drip-fed
    into later steps' jb streams; output cast to bf16 (host sums the two
    partial products per batch in f32) and DMAed.
  - input DMAs are ordered by first use and spread over the sync/gpsimd
    HWDGE queues (first-needed chunks first; one combined weights DMA);
    the scalar queue carries no DMA until the exp stream ends.
"""

import numpy as np
import ml_dtypes

EMBED = 1024
HEADS = 16
HD = 64  # head dim
N_CORES = 8
HPC = 8  # heads per core
GCOLS = HPC * HD  # 512 embed columns per core


def ts(i, n):
    return slice(i * n, (i + 1) * n)


def build_program(S):
    import concourse.bass as bass
    import concourse.tile as tile
    import concourse.mybir as mybir
    from concourse import bacc

    from concourse.alu_op_type import AluOpType

    f32 = mybir.dt.float32
    bf16 = mybir.dt.bfloat16
    i16 = mybir.dt.int16
    EXP = mybir.ActivationFunctionType.Exp

    NPAIR = 4  # head-pair blocks (2 heads each)
    NJB = S // 128  # key blocks
    NCCS = S // 512  # 512-wide query chunks
    NSTEP = NCCS * NPAIR  # (cc, pr) steps

    # Key blocks whose exp runs on the Vector engine via the Schraudolph
    # bit trick: bf16(exp(x)) ~= bitcast_bf16(int16(x*128/ln2 + 127*128-c)).
    # |rel err| <= ~4.5% per element, zero-mean; end-to-end (numpy sim on the
    # real inputs) max rel err 0.014 vs the 0.02 gate.  Offloading 5/16 of
    # the exp stream takes the Scalar engine off the critical path.
    # (Splitting EVERY tile between both engines was tried and is SLOWER --
    # 351us vs 322us -- the doubled per-op count pays a DVE pipe-DRAIN and
    # extra semaphores per tile.)
    DVE_JBS = {2, 5, 8, 11, 14}
    SCH_A = float(128.0 / np.log(2.0))
    SCH_B = float(127.0 * 128.0 - 7.4)

    nc = bacc.Bacc("TRN2", target_bir_lowering=False, debug=False)

    xqT_d = nc.dram_tensor("xqT", [NPAIR, 128, S], bf16, kind="ExternalInput").ap()
    xkT_d = nc.dram_tensor("xkT", [NPAIR, 128, S], bf16, kind="ExternalInput").ap()
    xvT_d = nc.dram_tensor("xvT", [NPAIR, 128, S], bf16, kind="ExternalInput").ap()
    wqkv_d = nc.dram_tensor("wqkv", [128, 3 * HD], bf16, kind="ExternalInput").ap()
    wo_d = nc.dram_tensor("woT", [NPAIR, 128, EMBED], bf16, kind="ExternalInput").ap()
    out_d = nc.dram_tensor("out", [S, EMBED], bf16, kind="ExternalOutput").ap()
    rec_dram = nc.dram_tensor("rec_scratch", [NPAIR, 2, S], f32)

    with tile.TileContext(nc) as tc:
        import contextlib

        with contextlib.ExitStack() as ctx:
            # ---- persistent pools ----
            const_p = ctx.enter_context(tc.tile_pool(name="const", bufs=1))
            qkv_p = ctx.enter_context(tc.tile_pool(name="qkv", bufs=1))
            vp_p = ctx.enter_context(tc.tile_pool(name="vp", bufs=1))
            ctxT_p = ctx.enter_context(tc.tile_pool(name="ctxT", bufs=1))
            wo_p = ctx.enter_context(tc.tile_pool(name="wo", bufs=1))
            # PSUM: "sc" slots 2 banks each (x2 = 4 banks), "cps" ctx
            # accumulators 1 bank (x2), "tr" transients 1 bank (x2) -> 8.
            sc_ps = ctx.enter_context(tc.tile_pool(name="sc", bufs=2, space="PSUM"))
            cps_ps = ctx.enter_context(tc.tile_pool(name="cps", bufs=2, space="PSUM"))
            tr_ps = ctx.enter_context(tc.tile_pool(name="tr", bufs=2, space="PSUM"))

            exp_p = ctx.enter_context(tc.tile_pool(name="exp", bufs=24))
            rec_p = ctx.enter_context(tc.tile_pool(name="rec", bufs=1))
            cu_p = ctx.enter_context(tc.tile_pool(name="cu", bufs=4))
            rrs_p = ctx.enter_context(tc.tile_pool(name="rrs", bufs=2))
            fco_p = ctx.enter_context(tc.tile_pool(name="fco", bufs=2))

            inv_sqrt_e = 1.0 / float(np.sqrt(EMBED))

            # ---- weights: one combined DMA (each trigger costs ~600ns of
            # queue time, so fewer, earlier triggers -> earlier first exp) --
            wqkv_s = const_p.tile([128, 3 * HD], bf16, tag="wqkv")
            nc.sync.dma_start(wqkv_s[:], wqkv_d[:])
            wq_s = wqkv_s[:, 0 * HD : 1 * HD]
            wk_s = wqkv_s[:, 1 * HD : 2 * HD]
            wv_s = wqkv_s[:, 2 * HD : 3 * HD]
            wo_t = [wo_p.tile([128, EMBED], bf16, tag=f"wo{p}", name=f"wo{p}") for p in range(NPAIR)]
            qT = [qkv_p.tile([128, S], bf16, tag=f"qT{p}", name=f"qT{p}") for p in range(NPAIR)]
            kT = [qkv_p.tile([128, S], bf16, tag=f"kT{p}", name=f"kT{p}") for p in range(NPAIR)]
            vp_t = [vp_p.tile([128, HPC * 65], bf16, tag=f"vp{jb}", name=f"vp{jb}") for jb in range(NJB)]
            ctxT = [ctxT_p.tile([128, S], bf16, tag=f"cx{p}", name=f"cx{p}") for p in range(NPAIR)]

            with tc.tile_pool(name="xin", bufs=1) as xin_p:
                xq = [xin_p.tile([128, S], bf16, tag=f"xq{p}", name=f"xq{p}") for p in range(NPAIR)]
                xk = [xin_p.tile([128, S], bf16, tag=f"xk{p}", name=f"xk{p}") for p in range(NPAIR)]
                xv = [xin_p.tile([128, S], bf16, tag=f"xv{p}", name=f"xv{p}") for p in range(NPAIR)]
                # DMA order = first use, packed so every piece lands before
                # the in-order engine queues reach its consumer.  Only the
                # sync and gpsimd queues carry input DMAs; the scalar queue
                # stays free for the exp stream.
                C = 512
                nc.sync.dma_start(xq[0][:, 0:C], xqT_d[0][:, 0:C])
                nc.gpsimd.dma_start(xk[0][:, 0:C], xkT_d[0][:, 0:C])
                nc.gpsimd.dma_start(xv[0][:], xvT_d[0])
                nc.sync.dma_start(xk[0][:, C:S], xkT_d[0][:, C:S])
                nc.gpsimd.dma_start(xv[1][:], xvT_d[1])
                nc.sync.dma_start(xq[1][:, 0:C], xqT_d[1][:, 0:C])
                nc.gpsimd.dma_start(xk[1][:], xkT_d[1])
                nc.sync.dma_start(xv[2][:], xvT_d[2])
                nc.gpsimd.dma_start(xv[3][:], xvT_d[3])
                nc.sync.dma_start(xq[2][:, 0:C], xqT_d[2][:, 0:C])
                nc.gpsimd.dma_start(xk[2][:], xkT_d[2])
                nc.sync.dma_start(xq[3][:, 0:C], xqT_d[3][:, 0:C])
                nc.gpsimd.dma_start(xk[3][:], xkT_d[3])
                for p in range(NPAIR):
                    nc.sync.dma_start(xq[p][:, C:S], xqT_d[p][:, C:S])
                for p in range(NPAIR):
                    (nc.sync if p % 2 else nc.gpsimd).dma_start(wo_t[p][:], wo_d[p])

                # ---- Q/K projection piece: qT[p][:, chunk] etc. ----
                def emit_qk(w_s, x_t, dst, p, ch, pool, tag):
                    ps = pool.tile([128, 512], f32, tag=tag)
                    for b in (0, 64):
                        nc.tensor.matmul(
                            ps[b : b + 64, 0:512],
                            lhsT=w_s[b : b + 64, :],
                            rhs=x_t[p][b : b + 64, ts(ch, 512)],
                            start=True,
                            stop=True,
                        )
                    nc.vector.tensor_copy(dst[p][:, ts(ch, 512)], ps[:, 0:512])

                # V-projection for one key block, one pair-half (pairs 2h..)
                # -> vp tile [V | ones] per head.  Split by half so the
                # early pieces only need xv0/xv1.
                def emit_vproj(jb, half):
                    pse = tr_ps.tile([128, 128], f32, tag="tr")
                    pso = tr_ps.tile([128, 128], f32, tag="tr")
                    for hh in range(4):
                        h = half * 4 + hh
                        p, b = h // 2, (h % 2) * 64
                        dst = pse if h % 2 == 0 else pso
                        nc.tensor.matmul(
                            dst[:, ts(hh // 2, 64)],
                            lhsT=xv[p][b : b + 64, ts(jb, 128)],
                            rhs=wv_s[b : b + 64, :],
                            start=True,
                            stop=True,
                        )
                    vpr = vp_t[jb].rearrange("p (h2 two c) -> p h2 two c", two=2, c=65)
                    h2s = slice(half * 2, half * 2 + 2)
                    pse_r = pse.rearrange("p (h c) -> p h c", c=64)
                    pso_r = pso.rearrange("p (h c) -> p h c", c=64)
                    nc.vector.tensor_copy(vpr[:, h2s, 0, 0:64], pse_r[:])
                    nc.vector.tensor_copy(vpr[:, h2s, 1, 0:64], pso_r[:])
                    nc.vector.memset(vpr[:, h2s, :, 64], 1.0)

                # drip list with deadlines (step whose scores/ctx need the
                # piece).  Ordered so each piece's input DMA has landed by
                # its emission slot; flushed before each step as a backstop.
                drip = []  # (deadline_step, emit_fn)
                # vproj half 0 (pairs 0/1, xv0/xv1): dripped in warm
                drip += [(1, lambda jb=jb: emit_vproj(jb, 0)) for jb in range(NJB)]
                for ch in range(4):
                    drip.append((1, lambda ch=ch: emit_qk(wk_s, xk, kT, 1, ch, tr_ps, "tr")))
                drip.append((1, lambda: emit_qk(wq_s, xq, qT, 1, 0, tr_ps, "tr")))
                # (vproj half 1 is emitted inline in the s==1 jb loop)
                for p in (2, 3):
                    for ch in range(4):
                        drip.append((p, lambda p=p, ch=ch: emit_qk(wk_s, xk, kT, p, ch, tr_ps, "tr")))
                    drip.append((p, lambda p=p: emit_qk(wq_s, xq, qT, p, 0, tr_ps, "tr")))
                for ch in (1, 2, 3):
                    for p in range(NPAIR):
                        drip.append((ch * 4 + p, lambda p=p, ch=ch: emit_qk(wq_s, xq, qT, p, ch, tr_ps, "tr")))

                # ---- scores+exp for one (step, jb) ----
                e_tiles = {}

                def emit_scores_exp(s, jb):
                    cc, pr = s // NPAIR, s % NPAIR
                    s_t = sc_ps.tile([128, 1024], f32, tag="sc")
                    for hl, b in ((0, 0), (1, 64)):
                        nc.tensor.matmul(
                            s_t[:, ts(hl, 512)],
                            lhsT=kT[pr][b : b + 64, ts(jb, 128)],
                            rhs=qT[pr][b : b + 64, ts(cc, 512)],
                            start=True,
                            stop=True,
                        )
                    e_t = exp_p.tile([128, 1024], bf16, tag="exp")
                    if jb in DVE_JBS:
                        nc.vector.tensor_scalar(
                            e_t[:].bitcast(i16),
                            s_t[:],
                            SCH_A * inv_sqrt_e,
                            SCH_B,
                            AluOpType.mult,
                            AluOpType.add,
                        )
                    else:
                        nc.scalar.activation(e_t[:], s_t[:], EXP, scale=inv_sqrt_e)
                    e_tiles[(s, jb)] = e_t

                # ---- warm: scores+exp for step 0, k-projections emitted
                # per chunk-group so the first exp starts as soon as the
                # first k/q chunk DMAs land.  Drip from jb>=8 (~16us in) so
                # the dripped work's inputs (xv0/1, xk1) have landed by the
                # time the in-order engine queues reach it.
                di = 0
                emit_qk(wq_s, xq, qT, 0, 0, sc_ps, "sc")
                for cg in range(4):
                    emit_qk(wk_s, xk, kT, 0, cg, sc_ps, "sc")
                    for jb in range(cg * 4, cg * 4 + 4):
                        emit_scores_exp(0, jb)
                        if 8 <= jb < 15:
                            for _ in range(3):
                                if di < len(drip):
                                    drip[di][1]()
                                    di += 1

                # ---- fc_out emission (one 512-col output block) ----
                def emit_fco(sb, oc, eng=None):
                    ps = tr_ps.tile([128, 512], f32, tag="tr")
                    for p in range(NPAIR):
                        nc.tensor.matmul(
                            ps[:],
                            lhsT=ctxT[p][:, ts(sb, 128)],
                            rhs=wo_t[p][:, ts(oc, 512)],
                            start=(p == 0),
                            stop=(p == NPAIR - 1),
                        )
                    fo = fco_p.tile([128, 512], bf16, tag="fco")
                    nc.vector.tensor_copy(fo[:], ps[:])
                    (eng or nc.sync).dma_start(out_d[ts(sb, 128), ts(oc, 512)], fo[:])

                # ---- normalization, split in two phases so neither the PE
                # nor the Act queue ever waits on the reciprocal chain.
                # Phase A (end of the step that finished the accumulators):
                # copy ctx+sum to SBUF, gather sum rows to partition 0,
                # reciprocal, stage to DRAM for the partition-broadcast.
                # Phase B (mid next step): broadcast-DMA + multiply.
                def emit_norm_a(s, cps_pair):
                    se_c = rec_p.tile([1, 1024], f32, tag="se")
                    rec_c = rec_p.tile([1, 1024], f32, tag="rec")
                    pr, cc = s % NPAIR, s // NPAIR
                    cus = []
                    for hl, cps in ((0, cps_pair[0]), (1, cps_pair[1])):
                        cu = cu_p.tile([65, 512], f32, tag="cu")
                        nc.vector.tensor_copy(cu[:], cps[:])
                        nc.sync.dma_start(se_c[0:1, ts(hl, 512)], cu[64:65, :])
                        cus.append(cu)
                    nc.vector.reciprocal_approx_fast(rec_c[:], se_c[:])
                    nc.sync.dma_start(rec_dram[pr][:, ts(cc, 512)], rec_c[:])
                    return cus

                def emit_norm_b(s, cus):
                    cc, pr = s // NPAIR, s % NPAIR
                    for hl in range(2):
                        rrs_c = rrs_p.tile([64, 512], f32, tag="rrs")
                        nc.sync.dma_start(
                            rrs_c[:],
                            rec_dram[pr][hl : hl + 1, ts(cc, 512)].partition_broadcast(64),
                        )
                        nc.vector.tensor_mul(
                            ctxT[pr][hl * 64 : hl * 64 + 64, ts(cc, 512)],
                            cus[hl][0:64, :],
                            rrs_c[:],
                        )

                # ---- main pipeline: step s emits exp(s) + ctx(s-1),
                # norm phase B of step s-2, and dripped proj/fc_out work ----
                fco_q = []
                normb = None  # (step, cus) pending phase B
                for s in range(1, NSTEP + 1):
                    # backstop: flush drip pieces this step's scores/ctx need
                    while di < len(drip) and drip[di][0] <= s:
                        drip[di][1]()
                        di += 1
                    pcc, ppr = (s - 1) // NPAIR, (s - 1) % NPAIR
                    cpsA = cps_ps.tile([65, 512], f32, tag="cps", name=f"cpsA{s - 1}")
                    cpsB = cps_ps.tile([65, 512], f32, tag="cps", name=f"cpsB{s - 1}")
                    for jb in range(NJB):
                        if s < NSTEP:
                            emit_scores_exp(s, jb)
                        if s == 1:
                            emit_vproj(jb, 1)
                        e_t = e_tiles.pop((s - 1, jb))
                        for hl, cps in ((0, cpsA), (1, cpsB)):
                            hh = ppr * 2 + hl
                            nc.tensor.matmul(
                                cps[:],
                                lhsT=vp_t[jb][:, hh * 65 : hh * 65 + 65],
                                rhs=e_t[:, ts(hl, 512)],
                                start=(jb == 0),
                                stop=(jb == NJB - 1),
                            )
                        if jb == 2 and normb is not None:
                            emit_norm_b(*normb)
                            if normb[0] % NPAIR == NPAIR - 1:
                                fcc = normb[0] // NPAIR
                                fco_q += [(fcc * 4 + g // 2, g % 2) for g in range(8)]
                            normb = None
                        if jb % 3 == 2:
                            if di < len(drip):
                                drip[di][1]()
                                di += 1
                            elif fco_q:
                                emit_fco(*fco_q.pop(0))
                            if fco_q and jb % 6 == 5:
                                emit_fco(*fco_q.pop(0))
                    cus = emit_norm_a(s - 1, (cpsA, cpsB))
                    normb = (s - 1, cus)
                # drain: phase B of the last step, then its fc_out chunk with
                # output DMA spread across sync/scalar/gpsimd queues
                emit_norm_b(*normb)
                fco_q += [((NCCS - 1) * 4 + g // 2, g % 2) for g in range(8)]
                engs = [nc.sync, nc.scalar, nc.gpsimd]
                for i, (sb, oc) in enumerate(fco_q):
                    emit_fco(sb, oc, engs[i % 3])

    nc.compile()
    return nc


def make_core_inputs(values, keys, queries, Wv, Wk, Wq, Wo, n, g, S):
    """Host-side marshaling for core (n, g): transpose + cast input slices."""
    bf = ml_dtypes.bfloat16
    cols = slice(g * GCOLS, (g + 1) * GCOLS)
    NPAIR = 4

    def xt(x):
        t = np.ascontiguousarray(x[n][:, cols].T.astype(bf))  # (512, S)
        return t.reshape(NPAIR, 128, S)

    def wstack(w):
        wt = w.T.astype(bf)  # (64, 64)
        return np.ascontiguousarray(np.concatenate([wt, wt], axis=0))  # (128, 64)

    woT = np.ascontiguousarray(Wo[:, cols].T.astype(bf)).reshape(
        NPAIR, 128, EMBED
    )
    return {
        "xqT": xt(queries),
        "xkT": xt(keys),
        "xvT": xt(values),
        "wqkv": np.ascontiguousarray(
            np.concatenate([wstack(Wq), wstack(Wk), wstack(Wv)], axis=1)
        ),
        "woT": woT,
    }


_PROG_CACHE = {}
TRACE = False
LAST_RESULTS = None


def kernel(values, keys, queries, mask, Wv, Wk, Wq, Wo, bo):
    global LAST_RESULTS
    from concourse.bass_utils import run_bass_kernel_spmd

    values = np.asarray(values, np.float32)
    keys = np.asarray(keys, np.float32)
    queries = np.asarray(queries, np.float32)
    Wv = np.asarray(Wv, np.float32)
    Wk = np.asarray(Wk, np.float32)
    Wq = np.asarray(Wq, np.float32)
    Wo = np.asarray(Wo, np.float32)
    bo = np.asarray(bo, np.float32)

    N, S, _ = queries.shape
    if S not in _PROG_CACHE:
        _PROG_CACHE[S] = build_program(S)
    nc = _PROG_CACHE[S]

    in_maps = [
        make_core_inputs(values, keys, queries, Wv, Wk, Wq, Wo, c // 2, c % 2, S)
        for c in range(N_CORES)
    ]
    res = run_bass_kernel_spmd(
        nc, in_maps, core_ids=list(range(N_CORES)), trace=TRACE
    )
    LAST_RESULTS = res
    out = np.empty((N, S, EMBED), np.float32)
    for n in range(N):
        out[n] = (
            res.results[2 * n]["out"].astype(np.float32)
            + res.results[2 * n + 1]["out"].astype(np.float32)
            + bo
        )
    return out
